# revision 12
# baseline (speedup 1.0000x reference)
"""BoxRenderLoss Trainium2 kernel (v2: separable prep + selector-matmul combine).

loss = mean over (box, fragment) pairs of masked min-squared-distance between
each box's 10x10 fragment grid and the other box's 100-point sampled boundary,
both directions, / (2*B*FP).

Key structure: fragment f = (i, j) is separable — every per-(f, row) quantity
depends only on (i, row) [x axis] or (j, row) [y axis].  So prep runs at
[80, 256] (partitions = (axis, i, rh), cols = rl; row r = rh*256+rl) instead
of the naive [100, 1024], a ~10x cut in elementwise work:

  u = gx*w + dx, v = -gx*w + dv, t = u*(24/tw), s = tw/24   (one K=16 matmul
  pair: lhsT = [gx;1] axis/rh selectors, rhs = host-packed per-row constants)
  k* = clamp(round(t), 0, 24) via the 2^23 trick; val = u - s*k*
  AX = min(u^2, v^2) [x], QX = valx^2, px = [min(u,v) >= 0], same for y.

Combine back to [100 fragments, 256 rows] per rh-chunk with ONE bf16 matmul
per (E1P, E2): lhsT = 0/1 selector [80, 100] replicating i-rows/j-rows,
E1 = AX_i + QY_j, P = px_i + py_j (fused in one N=512 matmul), E2 = QX_i+AY_j.
mask = [P < 1.5] (px+py < 2 == NOT inside both), so the whole reference
reduces to dmin = min(E1, E2); scr = (P<1.5)*dmin with accum_out row sums.
A final ones^T matmul collapses partitions so the output DMA is 16 bytes.
"""

import os
import numpy as np

# Exact float32 bit patterns of jnp.linspace(0.0, 1.0, 10) (fragment grid).
_LIN10 = np.array(
    [0, 1038323257, 1046711865, 1051372203, 1055100473,
     1057896676, 1059760811, 1061624946, 1063489081, 1065353216],
    dtype=np.uint32,
).view(np.float32)

_B = 4096
_FP = 100
_N_CORES = 8
_BPC = _B // _N_CORES        # 512 boxes per core
_R = 2 * _BPC                # 1024 virtual rows per core (2 directions)
_NRH = 4                     # row chunks
_RL = _R // _NRH             # 256 rows per chunk
_MAGIC = 8388608.0           # 2^23 round-to-nearest trick

LAST_RESULTS = None  # BassKernelResults of the most recent run (for test.py)

_compiled = {}


def _build_nc():
    import concourse.bass as bass
    import concourse.bacc as bacc
    import concourse.tile as tile
    from concourse import mybir

    f32 = mybir.dt.float32
    f32r = mybir.dt.float32r
    bf16 = mybir.dt.bfloat16
    Op = mybir.AluOpType
    Act = mybir.ActivationFunctionType

    nc = bacc.Bacc("TRN2", target_bir_lowering=False, debug=False,
                   num_devices=_N_CORES)
    mmin_d = nc.dram_tensor("mmin", [16, 1024], f32r, kind="ExternalInput").ap()
    lt_d = nc.dram_tensor("lt", [16, 104], f32r, kind="ExternalInput").ap()
    sel_d = nc.dram_tensor("sel", [104, 400], bf16, kind="ExternalInput").ap()
    out_d = nc.dram_tensor("out", [1, 4], f32, kind="ExternalOutput").ap()

    with tile.TileContext(nc) as tc:
        with (
            tc.tile_pool(name="const", bufs=1) as const,
            tc.tile_pool(name="sb", bufs=1) as sb,
            tc.tile_pool(name="sb2", bufs=2) as sb2,
            tc.tile_pool(name="ps", bufs=1, space="PSUM") as ps,
            tc.tile_pool(name="ps2", bufs=3, space="PSUM") as ps2,
            tc.tile_pool(name="ps3", bufs=2, space="PSUM") as ps3,
        ):
            mm_t = const.tile([16, 1024], f32r)
            nc.sync.dma_start(mm_t[:], mmin_d[:])
            lt_t = const.tile([16, 104], f32r)
            nc.scalar.dma_start(lt_t[:], lt_d[:])
            sel_t = const.tile([104, 400], bf16)
            nc.gpsimd.dma_start(sel_t[:], sel_d[:])
            ones_t = const.tile([100, 1], f32)
            nc.gpsimd.memset(ones_t[:], 1.0)
            part = const.tile([100, _NRH], f32)

            # Prep affine: [80, (q in u,v,t,sb) x 256] = lt^T @ mmin.
            pps = ps.tile([104, 1024], f32, tag="pps")
            nc.tensor.matmul(pps[:, 0:512], lt_t[:], mm_t[:, 0:512])
            nc.tensor.matmul(pps[:, 512:1024], lt_t[:], mm_t[:, 512:1024])
            U = pps[:, 0:256]
            V = pps[:, 256:512]
            Tq = pps[:, 512:768]
            SBq = pps[:, 768:1024]

            rhs1 = sb.tile([104, 512], bf16, tag="rhs1")
            rhs2 = sb.tile([104, 256], bf16, tag="rhs2")

            # k* path first: relu gates the kc->sk->val chain.
            r1 = sb.tile([104, 256], bf16, tag="r1")
            nc.scalar.activation(r1[:], Tq, Act.Relu)

            # Mask path + squares: depend only on the first prep matmul.
            pu = sb.tile([104, 256], bf16, tag="pu")
            nc.vector.tensor_scalar(pu[:], U, 0.0, None, Op.is_ge)
            pv = sb.tile([104, 256], bf16, tag="pv")
            nc.vector.tensor_scalar(pv[:], V, 0.0, None, Op.is_ge)
            nc.gpsimd.tensor_tensor(rhs1[:, 256:512], pu[:], pv[:], Op.mult)
            usq = sb.tile([104, 256], bf16, tag="usq")
            nc.scalar.activation(usq[:], U, Act.Square)
            vsq = sb.tile([104, 256], bf16, tag="vsq")
            nc.scalar.activation(vsq[:], V, Act.Square)
            nc.vector.tensor_tensor(rhs1[0:64, 0:256], usq[0:64, :],
                                    vsq[0:64, :], Op.min)
            nc.vector.tensor_tensor(rhs2[64:104, :], usq[64:104, :],
                                    vsq[64:104, :], Op.min)

            kc = sb.tile([104, 256], f32, tag="kc")
            nc.vector.tensor_scalar(kc[:], r1[:], _MAGIC, _MAGIC + 24.0,
                                    Op.add, Op.min)
            sk = sb.tile([104, 256], f32, tag="sk")
            nc.vector.scalar_tensor_tensor(sk[:], kc[:], _MAGIC, SBq,
                                           Op.subtract, Op.mult)
            val = sb.tile([104, 256], f32, tag="val")
            nc.vector.tensor_tensor(val[:], U, sk[:], Op.subtract)
            nc.scalar.activation(rhs1[64:104, 0:256], val[64:104, :], Act.Square)
            nc.vector.tensor_tensor(rhs2[0:64, :], val[0:64, :], val[0:64, :],
                                    Op.mult)

            for rh in range(_NRH):
                lhs = sel_t[:, rh * 100:(rh + 1) * 100]
                e1p = ps2.tile([100, 512], f32, tag="e1p")
                nc.tensor.matmul(e1p[:], lhs, rhs1[:])
                e2 = ps3.tile([100, 256], f32, tag="e2")
                nc.tensor.matmul(e2[:], lhs, rhs2[:])
                cp2 = sb2.tile([100, 256], bf16, tag="cp2")
                nc.scalar.activation(cp2[:], e2[:], Act.Copy)
                dmin = sb2.tile([100, 256], bf16, tag="dmin")
                nc.vector.tensor_tensor(dmin[:], e1p[:, 0:256], cp2[:], Op.min)
                scr = sb2.tile([100, 256], bf16, tag="scr")
                nc.vector.scalar_tensor_tensor(
                    scr[:], e1p[:, 256:512], 1.5, dmin[:], Op.is_lt, Op.mult,
                    accum_out=part[:, rh:rh + 1])

            fin = ps.tile([1, _NRH], f32, tag="fin")
            nc.tensor.matmul(fin[:], ones_t[:], part[:])
            fin_sb = const.tile([1, _NRH], f32)
            nc.vector.tensor_scalar(fin_sb[:], fin[:], 0.0, None, Op.add)
            nc.sync.dma_start(out_d[:], fin_sb[:])
    nc.compile()
    return nc


def _axis_cols(A, T, lo, hi):
    """Per-row combo vectors for one axis (lo/hi = coord idx, e.g. 0/2)."""
    w = A[:, hi] - A[:, lo]
    d = A[:, lo] - T[:, lo]
    dv = T[:, hi] - A[:, lo]
    tw = T[:, hi] - T[:, lo]
    with np.errstate(divide="ignore"):
        ri = np.where(tw != 0, np.float32(24.0) / tw,
                      np.float32(0.0)).astype(np.float32)
    s = tw / np.float32(24.0)
    return w, d, -w, dv, w * ri, d * ri, s


def _consts():
    """Input-independent tensors: prep lhsT [16, 104] f32, selectors [104,400] bf16."""
    import ml_dtypes
    lt = np.zeros((16, 104), dtype=np.float32)
    for a in range(2):
        for i in range(10):
            for rh in range(_NRH):
                p = a * 64 + i * 4 + rh
                lt[a * 4 + rh, p] = _LIN10[i]
                lt[8 + a * 4 + rh, p] = 1.0
    sel = np.zeros((104, 400), dtype=np.float32)
    for rh in range(_NRH):
        for f in range(100):
            i, j = f // 10, f % 10
            sel[i * 4 + rh, rh * 100 + f] = 1.0
            sel[64 + j * 4 + rh, rh * 100 + f] = 1.0
    return lt, sel.astype(ml_dtypes.bfloat16)


def _mmin_for_core(boxes_c, targets_c):
    """Host-packed per-row affine constants [16, 1024] f32 for one core."""
    boxes_c = boxes_c.astype(np.float32, copy=False)
    targets_c = targets_c.astype(np.float32, copy=False)
    mmin = np.zeros((16, 1024), dtype=np.float32)
    for a, (lo, hi) in enumerate(((0, 2), (1, 3))):
        o1 = _axis_cols(boxes_c, targets_c, lo, hi)    # dir1
        o2 = _axis_cols(targets_c, boxes_c, lo, hi)    # dir2
        w, d, nw, dv, wr, dr, s = (
            np.concatenate([x1, x2]).astype(np.float32)
            for x1, x2 in zip(o1, o2))
        z = np.zeros_like(w)
        M = {0: (w, nw, wr, z), 1: (d, dv, dr, s)}     # [c][q in u,v,t,sb]
        for c in range(2):
            for rh in range(_NRH):
                k = c * 8 + a * 4 + rh
                for q in range(4):
                    mmin[k, q * 256:(q + 1) * 256] = \
                        M[c][q][rh * _RL:(rh + 1) * _RL]
    return mmin


def kernel(boxes: np.ndarray, targets: np.ndarray) -> np.ndarray:
    from concourse.bass_utils import run_bass_kernel_spmd

    global LAST_RESULTS
    boxes = np.ascontiguousarray(boxes, dtype=np.float32)
    targets = np.ascontiguousarray(targets, dtype=np.float32)
    assert boxes.shape == (_B, 4) and targets.shape == (_B, 4)

    if "nc" not in _compiled:
        _compiled["nc"] = _build_nc()
    nc = _compiled["nc"]

    lt, sel = _consts()
    in_maps = []
    for c in range(_N_CORES):
        rows = slice(c * _BPC, (c + 1) * _BPC)
        in_maps.append({
            "mmin": _mmin_for_core(boxes[rows], targets[rows]),
            "lt": lt,
            "sel": sel,
        })

    trace = bool(int(os.environ.get("BOXLOSS_TRACE", "0")))
    res = run_bass_kernel_spmd(nc, in_maps, list(range(_N_CORES)),
                               trace=trace)
    LAST_RESULTS = res

    total = np.float64(0.0)
    for r in res.results:
        total += r["out"].astype(np.float64).sum()
    loss = total / (2.0 * _B * _FP)
    return np.array(loss, dtype=np.float32)


# revision 14
# speedup vs baseline: 1.0626x; 1.0626x over previous
"""BoxRenderLoss Trainium2 kernel (v2: separable prep + selector-matmul combine).

loss = mean over (box, fragment) pairs of masked min-squared-distance between
each box's 10x10 fragment grid and the other box's 100-point sampled boundary,
both directions, / (2*B*FP).

Key structure: fragment f = (i, j) is separable — every per-(f, row) quantity
depends only on (i, row) [x axis] or (j, row) [y axis].  So prep runs at
[80, 256] (partitions = (axis, i, rh), cols = rl; row r = rh*256+rl) instead
of the naive [100, 1024], a ~10x cut in elementwise work:

  u = gx*w + dx, v = -gx*w + dv, t = u*(24/tw), s = tw/24   (one K=16 matmul
  pair: lhsT = [gx;1] axis/rh selectors, rhs = host-packed per-row constants)
  k* = clamp(round(t), 0, 24) via the 2^23 trick; val = u - s*k*
  AX = min(u^2, v^2) [x], QX = valx^2, px = [min(u,v) >= 0], same for y.

Combine back to [100 fragments, 256 rows] per rh-chunk with ONE bf16 matmul
per (E1P, E2): lhsT = 0/1 selector [80, 100] replicating i-rows/j-rows,
E1 = AX_i + QY_j, P = px_i + py_j (fused in one N=512 matmul), E2 = QX_i+AY_j.
mask = [P < 1.5] (px+py < 2 == NOT inside both), so the whole reference
reduces to dmin = min(E1, E2); scr = (P<1.5)*dmin with accum_out row sums.
A final ones^T matmul collapses partitions so the output DMA is 16 bytes.
"""

import os
import numpy as np

# Exact float32 bit patterns of jnp.linspace(0.0, 1.0, 10) (fragment grid).
_LIN10 = np.array(
    [0, 1038323257, 1046711865, 1051372203, 1055100473,
     1057896676, 1059760811, 1061624946, 1063489081, 1065353216],
    dtype=np.uint32,
).view(np.float32)

_B = 4096
_FP = 100
_N_CORES = 8
_BPC = _B // _N_CORES        # 512 boxes per core
_R = 2 * _BPC                # 1024 virtual rows per core (2 directions)
_NRH = 4                     # row chunks
_RL = _R // _NRH             # 256 rows per chunk
_MAGIC = 8388608.0           # 2^23 round-to-nearest trick

LAST_RESULTS = None  # BassKernelResults of the most recent run (for test.py)

_compiled = {}


def _build_nc():
    import concourse.bass as bass
    import concourse.bacc as bacc
    import concourse.tile as tile
    from concourse import mybir

    f32 = mybir.dt.float32
    f32r = mybir.dt.float32r
    bf16 = mybir.dt.bfloat16
    Op = mybir.AluOpType
    Act = mybir.ActivationFunctionType

    nc = bacc.Bacc("TRN2", target_bir_lowering=False, debug=False,
                   num_devices=_N_CORES)
    mmin_d = nc.dram_tensor("mmin", [16, 1024], f32r, kind="ExternalInput").ap()
    lt_d = nc.dram_tensor("lt", [16, 104], f32r, kind="ExternalInput").ap()
    sel_d = nc.dram_tensor("sel", [104, 400], bf16, kind="ExternalInput").ap()
    out_d = nc.dram_tensor("out", [1, 4], f32, kind="ExternalOutput").ap()

    with tile.TileContext(nc) as tc:
        with (
            tc.tile_pool(name="const", bufs=1) as const,
            tc.tile_pool(name="sb", bufs=1) as sb,
            tc.tile_pool(name="sb2", bufs=2) as sb2,
            tc.tile_pool(name="ps", bufs=1, space="PSUM") as ps,
            tc.tile_pool(name="ps2", bufs=2, space="PSUM") as ps2,
            tc.tile_pool(name="ps3", bufs=2, space="PSUM") as ps3,
        ):
            mm_t = const.tile([16, 1024], f32r)
            nc.sync.dma_start(mm_t[:], mmin_d[:])
            lt_t = const.tile([16, 104], f32r)
            nc.scalar.dma_start(lt_t[:], lt_d[:])
            sel_t = const.tile([104, 400], bf16)
            nc.gpsimd.dma_start(sel_t[:], sel_d[:])
            ones_t = const.tile([100, 1], f32)
            nc.gpsimd.memset(ones_t[:], 1.0)
            part = const.tile([100, _NRH], f32)

            # Prep affine. Three matmuls: TSB first (gates the k* chain),
            # then U|V twice -- one copy per reader engine, because Tile
            # serializes cross-engine readers of the same PSUM tile.
            tsb = ps.tile([104, 512], f32, tag="tsb")
            nc.tensor.matmul(tsb[:], lt_t[:], mm_t[:, 512:1024])
            uv1 = ps.tile([104, 512], f32, tag="uv1")
            nc.tensor.matmul(uv1[:], lt_t[:], mm_t[:, 0:512])
            uv2 = ps.tile([104, 512], f32, tag="uv2")
            nc.tensor.matmul(uv2[:], lt_t[:], mm_t[:, 0:512])
            U1 = uv1[:, 0:256]
            V1 = uv1[:, 256:512]
            U2 = uv2[:, 0:256]
            V2 = uv2[:, 256:512]
            Tq = tsb[:, 0:256]
            SBq = tsb[:, 256:512]

            rhs1 = sb.tile([104, 512], bf16, tag="rhs1")
            rhs2 = sb.tile([104, 256], bf16, tag="rhs2")

            # k* path: relu (ACT) gates the kc->sk->val chain (DVE).
            r1 = sb.tile([104, 256], bf16, tag="r1")
            nc.scalar.activation(r1[:], Tq, Act.Relu)

            # Mask path on DVE (uv1) and squares on ACT (uv2), in parallel.
            pu = sb.tile([104, 256], bf16, tag="pu")
            nc.vector.tensor_scalar(pu[:], U1, 0.0, None, Op.is_ge)
            pv = sb.tile([104, 256], bf16, tag="pv")
            nc.vector.tensor_scalar(pv[:], V1, 0.0, None, Op.is_ge)
            nc.gpsimd.tensor_tensor(rhs1[:, 256:512], pu[:], pv[:], Op.mult)
            usq = sb.tile([104, 256], bf16, tag="usq")
            nc.scalar.activation(usq[:], U2, Act.Square)
            vsq = sb.tile([104, 256], bf16, tag="vsq")
            nc.scalar.activation(vsq[:], V2, Act.Square)

            kc = sb.tile([104, 256], f32, tag="kc")
            nc.vector.tensor_scalar(kc[:], r1[:], _MAGIC, _MAGIC + 24.0,
                                    Op.add, Op.min)
            sk = sb.tile([104, 256], f32, tag="sk")
            nc.vector.scalar_tensor_tensor(sk[:], kc[:], _MAGIC, SBq,
                                           Op.subtract, Op.mult)
            val = sb.tile([104, 256], f32, tag="val")
            nc.vector.tensor_tensor(val[:], U1, sk[:], Op.subtract)
            nc.vector.tensor_tensor(rhs1[0:64, 0:256], usq[0:64, :],
                                    vsq[0:64, :], Op.min)
            nc.vector.tensor_tensor(rhs2[64:104, :], usq[64:104, :],
                                    vsq[64:104, :], Op.min)
            nc.scalar.activation(rhs1[64:104, 0:256], val[64:104, :], Act.Square)
            nc.vector.tensor_tensor(rhs2[0:64, :], val[0:64, :], val[0:64, :],
                                    Op.mult)

            for rh in range(_NRH):
                lhs = sel_t[:, rh * 100:(rh + 1) * 100]
                e1p = ps2.tile([100, 512], f32, tag="e1p")
                nc.tensor.matmul(e1p[:], lhs, rhs1[:])
                e2 = ps3.tile([100, 256], f32, tag="e2")
                nc.tensor.matmul(e2[:], lhs, rhs2[:])
                cp2 = sb2.tile([100, 256], bf16, tag="cp2")
                nc.scalar.activation(cp2[:], e2[:], Act.Copy)
                dmin = sb2.tile([100, 256], bf16, tag="dmin")
                nc.vector.tensor_tensor(dmin[:], e1p[:, 0:256], cp2[:], Op.min)
                scr = sb2.tile([100, 256], bf16, tag="scr")
                nc.vector.scalar_tensor_tensor(
                    scr[:], e1p[:, 256:512], 1.5, dmin[:], Op.is_lt, Op.mult,
                    accum_out=part[:, rh:rh + 1])

            fin = ps.tile([1, _NRH], f32, tag="fin")
            nc.tensor.matmul(fin[:], ones_t[:], part[:])
            fin_sb = const.tile([1, _NRH], f32)
            nc.vector.tensor_scalar(fin_sb[:], fin[:], 0.0, None, Op.add)
            nc.sync.dma_start(out_d[:], fin_sb[:])
    nc.compile()
    return nc


def _axis_cols(A, T, lo, hi):
    """Per-row combo vectors for one axis (lo/hi = coord idx, e.g. 0/2)."""
    w = A[:, hi] - A[:, lo]
    d = A[:, lo] - T[:, lo]
    dv = T[:, hi] - A[:, lo]
    tw = T[:, hi] - T[:, lo]
    with np.errstate(divide="ignore"):
        ri = np.where(tw != 0, np.float32(24.0) / tw,
                      np.float32(0.0)).astype(np.float32)
    s = tw / np.float32(24.0)
    return w, d, -w, dv, w * ri, d * ri, s


def _consts():
    """Input-independent tensors: prep lhsT [16, 104] f32, selectors [104,400] bf16."""
    import ml_dtypes
    lt = np.zeros((16, 104), dtype=np.float32)
    for a in range(2):
        for i in range(10):
            for rh in range(_NRH):
                p = a * 64 + i * 4 + rh
                lt[a * 4 + rh, p] = _LIN10[i]
                lt[8 + a * 4 + rh, p] = 1.0
    sel = np.zeros((104, 400), dtype=np.float32)
    for rh in range(_NRH):
        for f in range(100):
            i, j = f // 10, f % 10
            sel[i * 4 + rh, rh * 100 + f] = 1.0
            sel[64 + j * 4 + rh, rh * 100 + f] = 1.0
    return lt, sel.astype(ml_dtypes.bfloat16)


def _mmin_for_core(boxes_c, targets_c):
    """Host-packed per-row affine constants [16, 1024] f32 for one core."""
    boxes_c = boxes_c.astype(np.float32, copy=False)
    targets_c = targets_c.astype(np.float32, copy=False)
    mmin = np.zeros((16, 1024), dtype=np.float32)
    for a, (lo, hi) in enumerate(((0, 2), (1, 3))):
        o1 = _axis_cols(boxes_c, targets_c, lo, hi)    # dir1
        o2 = _axis_cols(targets_c, boxes_c, lo, hi)    # dir2
        w, d, nw, dv, wr, dr, s = (
            np.concatenate([x1, x2]).astype(np.float32)
            for x1, x2 in zip(o1, o2))
        z = np.zeros_like(w)
        M = {0: (w, nw, wr, z), 1: (d, dv, dr, s)}     # [c][q in u,v,t,sb]
        for c in range(2):
            for rh in range(_NRH):
                k = c * 8 + a * 4 + rh
                for q in range(4):
                    mmin[k, q * 256:(q + 1) * 256] = \
                        M[c][q][rh * _RL:(rh + 1) * _RL]
    return mmin


def kernel(boxes: np.ndarray, targets: np.ndarray) -> np.ndarray:
    from concourse.bass_utils import run_bass_kernel_spmd

    global LAST_RESULTS
    boxes = np.ascontiguousarray(boxes, dtype=np.float32)
    targets = np.ascontiguousarray(targets, dtype=np.float32)
    assert boxes.shape == (_B, 4) and targets.shape == (_B, 4)

    if "nc" not in _compiled:
        _compiled["nc"] = _build_nc()
    nc = _compiled["nc"]

    lt, sel = _consts()
    in_maps = []
    for c in range(_N_CORES):
        rows = slice(c * _BPC, (c + 1) * _BPC)
        in_maps.append({
            "mmin": _mmin_for_core(boxes[rows], targets[rows]),
            "lt": lt,
            "sel": sel,
        })

    trace = bool(int(os.environ.get("BOXLOSS_TRACE", "0")))
    res = run_bass_kernel_spmd(nc, in_maps, list(range(_N_CORES)),
                               trace=trace)
    LAST_RESULTS = res

    total = np.float64(0.0)
    for r in res.results:
        total += r["out"].astype(np.float64).sum()
    loss = total / (2.0 * _B * _FP)
    return np.array(loss, dtype=np.float32)
